# revision 10
# baseline (speedup 1.0000x reference)
"""ATACSeq RBF-embedding kernel for 8 Trainium2 NeuronCores.

Math (per sample b with chromosome k = chrom[b]):
    w[n]  = exp(-(pos_b - centers[k,n])^2 / (2 * exp(logvar[k,n])))
    out_b = (w / w.sum()) @ embeddings[k]          # [D]

Sharding: samples are grouped by chromosome on the host; core i owns
chromosomes [3i, 3i+3) and receives ONLY those embedding matrices
(3 x 1 MB instead of the full 25 MB stack) plus its grouped, padded
positions.  All per-sample math (RBF weights, normalization, weighted
sum) runs on-device:

  - diff matmul (PE, K=2):  lhsT=[t_n; c_n*t_n], rhs=[p_b; -1] gives
    diff[n,b] = t_n*(p_b - c_n) with t = sqrt(1/(2v)) folded in -- one
    matmul fuses the partition broadcast, subtract and variance scale.
  - square (DVE) + exp (ACT, scale=-1) -> unnormalized w [n, b]
  - weighted sum:  w.T @ E accumulated over 4 partition chunks of n in
    PSUM, using float32r (full-rate fp32 matmul at N=512).
  - normalizer:    w.T @ ones -> Z [b, 1], reciprocal, and a per-
    partition tensor_scalar multiply of the PSUM result.

The host then scatters each core's [3, CAP, D] block back into the
full [B, D] output using the sort permutation.
"""

import math
import sys
import types

import numpy as np

import concourse.bass as bass
import concourse.tile as tile
from concourse import bacc, mybir
from concourse.bass_utils import run_bass_kernel_spmd


def _ensure_ntff_hook():
    """Provide antenv.axon_hooks if the container's antenv stub lacks it.

    bass_utils' BASS_TRACE=1 path imports antenv.axon_hooks to fetch the
    NTFF profile hook; the trimmed antenv in this container doesn't ship
    it.  Register a minimal holder backed by trn_agent_boot's ctypes
    shim.  Fully guarded: on any failure tracing is simply unavailable.
    """
    try:
        import antenv.axon_hooks  # noqa: F401

        return
    except Exception:
        pass
    try:
        import antenv

        mod = types.ModuleType("antenv.axon_hooks")
        holder = [None, False]

        def set_axon_ntff_profile_hook(h):
            holder[0] = h
            holder[1] = True

        def get_axon_ntff_profile_hook():
            if not holder[1]:
                holder[1] = True
                try:
                    from trn_agent_boot.trn_boot import (
                        _ntff_profile_via_ctypes,
                    )

                    holder[0] = _ntff_profile_via_ctypes(
                        "/opt/axon/libaxon_pjrt.so"
                    )
                except Exception:
                    holder[0] = None
            return holder[0]

        mod.set_axon_ntff_profile_hook = set_axon_ntff_profile_hook
        mod.get_axon_ntff_profile_hook = get_axon_ntff_profile_hook
        sys.modules["antenv.axon_hooks"] = mod
        antenv.axon_hooks = mod
    except Exception:
        pass


_ensure_ntff_hook()

N_CORES = 8
P = 128  # SBUF partitions

# Filled in by kernel() on every call so a harness/test can inspect the
# BassKernelResults of the last run (exec_time_ns etc. when BASS_TRACE=1).
LAST_RESULTS = None

_NC_CACHE = {}


def _build_nc(KPC: int, N: int, D: int, CAP: int, SC: int):
    """Build the SPMD Bass module (identical program for every core).

    Per-core DRAM I/O:
      emb [KPC, N, D] f32   core's embedding matrices
      tct [KPC, 2, N] f32   row0 = t = sqrt(1/(2v)), row1 = centers * t
      pos [KPC, CAP]  f32   grouped positions, padded with 0
      out [KPC, CAP, D] f32 per-sample outputs (padded rows are garbage)
    """
    f32 = mybir.dt.float32
    f32r = mybir.dt.float32r
    NCH = N // P          # chunks of the center dim (4 for N=512)
    n_sc = CAP // SC      # sample chunks of <=128 samples

    nc = bacc.Bacc("TRN2", target_bir_lowering=False, debug=False)
    emb = nc.dram_tensor("emb", [KPC, N, D], f32, kind="ExternalInput").ap()
    tct = nc.dram_tensor("tct", [KPC, 2, N], f32, kind="ExternalInput").ap()
    # row 0: positions, row 1: -1 (rhs of the rank-2 diff matmul)
    pos = nc.dram_tensor("pos", [KPC, 2, CAP], f32, kind="ExternalInput").ap()
    out = nc.dram_tensor("out", [KPC, CAP, D], f32, kind="ExternalOutput").ap()

    with tile.TileContext(nc) as tc:
        with (
            tc.tile_pool(name="consts", bufs=1) as consts,
            tc.tile_pool(name="embp", bufs=2) as embp,
            tc.tile_pool(name="wp", bufs=3) as wp,
            tc.tile_pool(name="small", bufs=3) as small,
            tc.tile_pool(name="ps_diff", bufs=2, space="PSUM") as ps_diff,
            tc.tile_pool(name="ps_out", bufs=2, space="PSUM") as ps_out,
        ):
            ones_col = consts.tile([P, 1], f32)
            nc.vector.memset(ones_col, 1.0)

            for k in range(KPC):
                # E[k] -> [128, NCH, D]; 1 MB contiguous-partition DMA.
                e_sb = embp.tile([P, NCH, D], f32, tag="e")
                nc.sync.dma_start(
                    out=e_sb, in_=emb[k].rearrange("(c p) d -> p c d", p=P)
                )
                tct_sb = small.tile([2, N], f32, tag="tct")
                nc.sync.dma_start(out=tct_sb, in_=tct[k])
                # rhs of the diff matmul: row0 = positions, row1 = -1
                pn = small.tile([2, CAP], f32, tag="pn")
                nc.sync.dma_start(out=pn, in_=pos[k])

                for c in range(n_sc):
                    # diff[n, b] = t_n * (p_b - c_n), 4 chunks of n side
                    # by side in one PSUM bank
                    diff = ps_diff.tile([P, NCH * SC], f32, tag="diff")
                    for j in range(NCH):
                        nc.tensor.matmul(
                            out=diff[:, j * SC : (j + 1) * SC],
                            lhsT=tct_sb[:, j * P : (j + 1) * P],
                            rhs=pn[:, c * SC : (c + 1) * SC],
                            start=True,
                            stop=True,
                        )
                    # DVE may read only one PSUM operand: copy out, then square
                    dsb = wp.tile([P, NCH * SC], f32, tag="dsb")
                    nc.vector.tensor_copy(dsb, diff)
                    sq = wp.tile([P, NCH * SC], f32, tag="sq")
                    nc.vector.tensor_mul(sq, dsb, dsb)
                    w = wp.tile([P, NCH * SC], f32, tag="w")
                    nc.scalar.activation(
                        out=w,
                        in_=sq,
                        func=mybir.ActivationFunctionType.Exp,
                        scale=-1.0,
                    )
                    o_ps = ps_out.tile([SC, D], f32, tag="o")
                    z_ps = ps_out.tile([SC, 1], f32, tag="z")
                    for j in range(NCH):
                        nc.tensor.matmul(
                            out=o_ps,
                            lhsT=w[:, j * SC : (j + 1) * SC],
                            rhs=e_sb[:, j, :],
                            start=(j == 0),
                            stop=(j == NCH - 1),
                        )
                    for j in range(NCH):
                        nc.tensor.matmul(
                            out=z_ps,
                            lhsT=w[:, j * SC : (j + 1) * SC],
                            rhs=ones_col,
                            start=(j == 0),
                            stop=(j == NCH - 1),
                        )
                    rz = small.tile([SC, 1], f32, tag="rz")
                    nc.vector.reciprocal(rz, z_ps)
                    o_sb = wp.tile([SC, D], f32, tag="osb")
                    nc.vector.tensor_scalar_mul(o_sb, o_ps, rz)
                    nc.sync.dma_start(
                        out=out[k, c * SC : (c + 1) * SC, :], in_=o_sb
                    )
    nc.compile()
    return nc


def _get_nc(KPC, N, D, CAP, SC):
    key = (KPC, N, D, CAP, SC)
    if key not in _NC_CACHE:
        _NC_CACHE[key] = _build_nc(*key)
    return _NC_CACHE[key]


def _shard(chromosome, position, embeddings, centers, log_variances):
    """Group samples by chromosome and build per-core input maps."""
    B = chromosome.shape[0]
    K, N, D = embeddings.shape
    KPC = math.ceil(K / N_CORES)
    Kpad = KPC * N_CORES

    counts = np.bincount(chromosome, minlength=Kpad)
    maxc = max(1, int(counts.max()))
    if maxc <= P:
        CAP = max(32, ((maxc + 31) // 32) * 32)
        SC = CAP
    else:
        CAP = ((maxc + P - 1) // P) * P
        SC = P

    order = np.argsort(chromosome, kind="stable")
    starts = np.zeros(Kpad + 1, dtype=np.int64)
    starts[1 : K + 1] = np.cumsum(counts[:K])
    starts[K + 1 :] = starts[K]
    sorted_pos = position[order, 0].astype(np.float32)

    pos_all = np.zeros((Kpad, 2, CAP), dtype=np.float32)
    pos_all[:, 1, :] = -1.0
    for k in range(Kpad):
        pos_all[k, 0, : counts[k]] = sorted_pos[starts[k] : starts[k + 1]]

    t = np.sqrt(0.5 * np.exp(-log_variances.astype(np.float64))).astype(
        np.float32
    )
    ct = (centers.astype(np.float32) * t).astype(np.float32)
    tct_all = np.zeros((Kpad, 2, N), dtype=np.float32)
    tct_all[:K, 0] = t
    tct_all[:K, 1] = ct

    emb_all = np.zeros((Kpad, N, D), dtype=np.float32)
    emb_all[:K] = embeddings

    in_maps = []
    for i in range(N_CORES):
        sl = slice(i * KPC, (i + 1) * KPC)
        in_maps.append(
            {
                "emb": np.ascontiguousarray(emb_all[sl]),
                "tct": np.ascontiguousarray(tct_all[sl]),
                "pos": np.ascontiguousarray(pos_all[sl]),
            }
        )
    meta = (B, D, KPC, CAP, SC, order, starts, counts)
    return in_maps, meta


def kernel(chromosome, position, embeddings, centers, log_variances):
    global LAST_RESULTS
    chromosome = np.asarray(chromosome, dtype=np.int32)
    position = np.asarray(position, dtype=np.float32)
    embeddings = np.asarray(embeddings, dtype=np.float32)
    centers = np.asarray(centers, dtype=np.float32)
    log_variances = np.asarray(log_variances, dtype=np.float32)

    in_maps, meta = _shard(
        chromosome, position, embeddings, centers, log_variances
    )
    B, D, KPC, CAP, SC, order, starts, counts = meta
    N = embeddings.shape[1]

    nc = _get_nc(KPC, N, D, CAP, SC)
    res = run_bass_kernel_spmd(nc, in_maps, core_ids=list(range(N_CORES)))
    LAST_RESULTS = res

    out_full = np.zeros((B, D), dtype=np.float32)
    for i in range(N_CORES):
        o = res.results[i]["out"]  # [KPC, CAP, D]
        for tloc in range(KPC):
            k = i * KPC + tloc
            if k >= len(counts) or counts[k] == 0:
                continue
            idx = order[starts[k] : starts[k + 1]]
            out_full[idx] = o[tloc, : counts[k]]
    return out_full


# revision 15
# speedup vs baseline: 1.2594x; 1.2594x over previous
"""ATACSeq RBF-embedding kernel for 8 Trainium2 NeuronCores.

Math (per sample b with chromosome k = chrom[b]):
    w[n]  = exp(-(pos_b - centers[k,n])^2 / (2 * exp(logvar[k,n])))
    out_b = (w / w.sum()) @ embeddings[k]          # [D]

Sharding: samples are grouped by chromosome on the host; core i owns
chromosomes [3i, 3i+3) and receives ONLY those embedding matrices
(3 x 1 MB instead of the full 25 MB stack) plus its grouped, padded
positions.  All per-sample math (RBF weights, normalization, weighted
sum) runs on-device:

  - diff matmul (PE, K=2):  lhsT=[t_n; c_n*t_n], rhs=[p_b; -1] gives
    diff[n,b] = t_n*(p_b - c_n) with t = sqrt(1/(2v)) folded in -- one
    matmul fuses the partition broadcast, subtract and variance scale.
  - square (DVE) + exp (ACT, scale=-1) -> unnormalized w [n, b]
  - weighted sum:  w.T @ E accumulated over 4 partition chunks of n in
    PSUM, using float32r (full-rate fp32 matmul at N=512).
  - normalizer:    w.T @ ones -> Z [b, 1], reciprocal, and a per-
    partition tensor_scalar multiply of the PSUM result.

The host then scatters each core's [3, CAP, D] block back into the
full [B, D] output using the sort permutation.
"""

import math
import sys
import types

import numpy as np

import concourse.bass as bass
import concourse.tile as tile
from concourse import bacc, mybir
from concourse.bass_utils import run_bass_kernel_spmd


def _ensure_ntff_hook():
    """Provide antenv.axon_hooks if the container's antenv stub lacks it.

    bass_utils' BASS_TRACE=1 path imports antenv.axon_hooks to fetch the
    NTFF profile hook; the trimmed antenv in this container doesn't ship
    it.  Register a minimal holder backed by trn_agent_boot's ctypes
    shim.  Fully guarded: on any failure tracing is simply unavailable.
    """
    try:
        import antenv.axon_hooks  # noqa: F401

        return
    except Exception:
        pass
    try:
        import antenv

        mod = types.ModuleType("antenv.axon_hooks")
        holder = [None, False]

        def set_axon_ntff_profile_hook(h):
            holder[0] = h
            holder[1] = True

        def get_axon_ntff_profile_hook():
            if not holder[1]:
                holder[1] = True
                try:
                    from trn_agent_boot.trn_boot import (
                        _ntff_profile_via_ctypes,
                    )

                    holder[0] = _ntff_profile_via_ctypes(
                        "/opt/axon/libaxon_pjrt.so"
                    )
                except Exception:
                    holder[0] = None
            return holder[0]

        mod.set_axon_ntff_profile_hook = set_axon_ntff_profile_hook
        mod.get_axon_ntff_profile_hook = get_axon_ntff_profile_hook
        sys.modules["antenv.axon_hooks"] = mod
        antenv.axon_hooks = mod
    except Exception:
        pass


_ensure_ntff_hook()

N_CORES = 8
P = 128  # SBUF partitions

# Filled in by kernel() on every call so a harness/test can inspect the
# BassKernelResults of the last run (exec_time_ns etc. when BASS_TRACE=1).
LAST_RESULTS = None

_NC_CACHE = {}


def _build_nc(KPC: int, N: int, D: int, CAP: int, SC: int):
    """Build the SPMD Bass module (identical program for every core).

    Per-core DRAM I/O:
      emb [KPC, N, D] f32   core's embedding matrices
      tct [KPC, 2, N] f32   row0 = t = sqrt(1/(2v)), row1 = centers * t
      pos [KPC, CAP]  f32   grouped positions, padded with 0
      out [KPC, CAP, D] f32 per-sample outputs (padded rows are garbage)
    """
    f32 = mybir.dt.float32
    f32r = mybir.dt.float32r
    NCH = N // P          # chunks of the center dim (4 for N=512)
    n_sc = CAP // SC      # sample chunks of <=128 samples

    nc = bacc.Bacc("TRN2", target_bir_lowering=False, debug=False)
    emb = nc.dram_tensor("emb", [KPC, N, D], f32r, kind="ExternalInput").ap()
    tct = nc.dram_tensor("tct", [KPC, 2, N], f32, kind="ExternalInput").ap()
    # row 0: positions, row 1: -1 (rhs of the rank-2 diff matmul)
    pos = nc.dram_tensor("pos", [KPC, 2, CAP], f32, kind="ExternalInput").ap()
    out = nc.dram_tensor("out", [KPC, CAP, D], f32, kind="ExternalOutput").ap()

    with tile.TileContext(nc) as tc:
        with (
            tc.tile_pool(name="consts", bufs=1) as consts,
            tc.tile_pool(name="embp", bufs=2) as embp,
            tc.tile_pool(name="wp", bufs=3) as wp,
            tc.tile_pool(name="small", bufs=3) as small,
            tc.tile_pool(name="ps_diff", bufs=2, space="PSUM") as ps_diff,
            tc.tile_pool(name="ps_out", bufs=2, space="PSUM") as ps_out,
        ):
            ones_col = consts.tile([P, 1], f32)
            nc.vector.memset(ones_col, 1.0)

            for k in range(KPC):
                # E[k] -> [128, NCH, D]; 1 MB contiguous-partition DMA.
                e_sb = embp.tile([P, NCH, D], f32r, tag="e")
                nc.sync.dma_start(
                    out=e_sb, in_=emb[k].rearrange("(c p) d -> p c d", p=P)
                )
                tct_sb = small.tile([2, N], f32, tag="tct")
                nc.sync.dma_start(out=tct_sb, in_=tct[k])
                # rhs of the diff matmul: row0 = positions, row1 = -1
                pn = small.tile([2, CAP], f32, tag="pn")
                nc.sync.dma_start(out=pn, in_=pos[k])

                for c in range(n_sc):
                    # diff[n, b] = t_n * (p_b - c_n), 4 chunks of n side
                    # by side in one PSUM bank
                    diff = ps_diff.tile([P, NCH * SC], f32, tag="diff")
                    for j in range(NCH):
                        nc.tensor.matmul(
                            out=diff[:, j * SC : (j + 1) * SC],
                            lhsT=tct_sb[:, j * P : (j + 1) * P],
                            rhs=pn[:, c * SC : (c + 1) * SC],
                            start=True,
                            stop=True,
                        )
                    # DVE may read only one PSUM operand: copy out, then square
                    dsb = wp.tile([P, NCH * SC], f32, tag="dsb")
                    nc.vector.tensor_copy(dsb, diff)
                    sq = wp.tile([P, NCH * SC], f32, tag="sq")
                    nc.vector.tensor_mul(sq, dsb, dsb)
                    # ACT writes float32r directly: the rounding producer
                    # the fp32r matmuls below require.
                    w = wp.tile([P, NCH * SC], f32r, tag="w")
                    nc.scalar.activation(
                        out=w,
                        in_=sq,
                        func=mybir.ActivationFunctionType.Exp,
                        scale=-1.0,
                    )
                    o_ps = ps_out.tile([SC, D], f32, tag="o")
                    z_ps = ps_out.tile([SC, 1], f32, tag="z")
                    for j in range(NCH):
                        nc.tensor.matmul(
                            out=o_ps,
                            lhsT=w[:, j * SC : (j + 1) * SC],
                            rhs=e_sb[:, j, :],
                            start=(j == 0),
                            stop=(j == NCH - 1),
                        )
                    for j in range(NCH):
                        nc.tensor.matmul(
                            out=z_ps,
                            lhsT=w[:, j * SC : (j + 1) * SC].bitcast(f32),
                            rhs=ones_col,
                            start=(j == 0),
                            stop=(j == NCH - 1),
                        )
                    rz = small.tile([SC, 1], f32, tag="rz")
                    nc.vector.reciprocal(rz, z_ps)
                    o_sb = wp.tile([SC, D], f32, tag="osb")
                    nc.vector.tensor_scalar_mul(o_sb, o_ps, rz)
                    nc.sync.dma_start(
                        out=out[k, c * SC : (c + 1) * SC, :], in_=o_sb
                    )
    nc.compile()
    return nc


def _get_nc(KPC, N, D, CAP, SC):
    key = (KPC, N, D, CAP, SC)
    if key not in _NC_CACHE:
        _NC_CACHE[key] = _build_nc(*key)
    return _NC_CACHE[key]


def _shard(chromosome, position, embeddings, centers, log_variances):
    """Group samples by chromosome and build per-core input maps."""
    B = chromosome.shape[0]
    K, N, D = embeddings.shape
    KPC = math.ceil(K / N_CORES)
    Kpad = KPC * N_CORES

    counts = np.bincount(chromosome, minlength=Kpad)
    maxc = max(1, int(counts.max()))
    if maxc <= P:
        CAP = max(32, ((maxc + 31) // 32) * 32)
        SC = CAP
    else:
        CAP = ((maxc + P - 1) // P) * P
        SC = P

    order = np.argsort(chromosome, kind="stable")
    starts = np.zeros(Kpad + 1, dtype=np.int64)
    starts[1 : K + 1] = np.cumsum(counts[:K])
    starts[K + 1 :] = starts[K]
    sorted_pos = position[order, 0].astype(np.float32)

    pos_all = np.zeros((Kpad, 2, CAP), dtype=np.float32)
    pos_all[:, 1, :] = -1.0
    for k in range(Kpad):
        pos_all[k, 0, : counts[k]] = sorted_pos[starts[k] : starts[k + 1]]

    t = np.sqrt(0.5 * np.exp(-log_variances.astype(np.float64))).astype(
        np.float32
    )
    ct = (centers.astype(np.float32) * t).astype(np.float32)
    tct_all = np.zeros((Kpad, 2, N), dtype=np.float32)
    tct_all[:K, 0] = t
    tct_all[:K, 1] = ct

    emb_all = np.zeros((Kpad, N, D), dtype=np.float32)
    emb_all[:K] = embeddings

    in_maps = []
    for i in range(N_CORES):
        sl = slice(i * KPC, (i + 1) * KPC)
        in_maps.append(
            {
                "emb": np.ascontiguousarray(emb_all[sl]),
                "tct": np.ascontiguousarray(tct_all[sl]),
                "pos": np.ascontiguousarray(pos_all[sl]),
            }
        )
    meta = (B, D, KPC, CAP, SC, order, starts, counts)
    return in_maps, meta


def kernel(chromosome, position, embeddings, centers, log_variances):
    global LAST_RESULTS
    chromosome = np.asarray(chromosome, dtype=np.int32)
    position = np.asarray(position, dtype=np.float32)
    embeddings = np.asarray(embeddings, dtype=np.float32)
    centers = np.asarray(centers, dtype=np.float32)
    log_variances = np.asarray(log_variances, dtype=np.float32)

    in_maps, meta = _shard(
        chromosome, position, embeddings, centers, log_variances
    )
    B, D, KPC, CAP, SC, order, starts, counts = meta
    N = embeddings.shape[1]

    nc = _get_nc(KPC, N, D, CAP, SC)
    res = run_bass_kernel_spmd(nc, in_maps, core_ids=list(range(N_CORES)))
    LAST_RESULTS = res

    out_full = np.zeros((B, D), dtype=np.float32)
    for i in range(N_CORES):
        o = res.results[i]["out"]  # [KPC, CAP, D]
        for tloc in range(KPC):
            k = i * KPC + tloc
            if k >= len(counts) or counts[k] == 0:
                continue
            idx = order[starts[k] : starts[k + 1]]
            out_full[idx] = o[tloc, : counts[k]]
    return out_full
